# revision 14
# baseline (speedup 1.0000x reference)
"""Edge-conditioned causal self-attention (Graphormer-style) on 8 Trainium2 cores.

Math (per batch b, head h; i = query pos, j = key pos; E=32, G=16 edge types):
  scores[i,j] = sum_e q[i,e] * k[j,e] * ektab[bm[j,i], e] / sqrt(E)
  attn        = softmax_j(scores + biastab[bm[i,j]])  (causal j<=i)
  y[i,e]      = sum_j attn[i,j] * v[j,e] * evtab[bm[j,i], e]
  out         = y @ w_proj
where ektab = edge_emb @ w_edge_k (per-head slice), evtab likewise, and bm is
the integer edge-type matrix.  NOTE: the additive bias indexes bm TRANSPOSED
relative to the k/v modulation (matches the reference einsums).

Since there are only G=16 edge types, scores decompose into 16 per-type Gram
planes S_g = (q .* ektab[g]) @ k^T.  The per-(j,i) selection and the additive
bias are both folded into host-precomputed f16 planes
  w_g[j,i] = (bm[j,i]==g & j<=i) * exp(biastab[bm[i,j]])
so the masked-biased softmax numerators are e2_g = w_g * exp(S_g), and the
output accumulates yT += (v .* evtab[g] | 1) @ e2_g with the ones column
giving the softmax denominator for free.

Schedule (engine-balanced for the TimelineSim cost model):
  - the h0/h1 head streams are interleaved batch-by-batch (shared one-bank
    yT PSUM tile: h0 rows 0-32, h1 rows 64-96) so the PE always has
    independent ready matmuls and stays at full clock;
  - S_g matmuls grouped into 3-bank PSUM tiles, exp'd in one batched strided
    activation (Scalar is the only engine with a cheap PSUM exit);
  - e2 multiplies split: bottom NG g's per batch on GPSIMD (per-plane TTs),
    rest on Vector (one wide strided TT per batch, 2x rate);
  - yT matmuls are emitted LAG batches late so the in-order PE sequencer
    never head-of-line blocks on a dependency-stalled yT matmul.
Sharding: core c -> batch b=c//4, head pair (2*(c%4), 2*(c%4)+1).  Fully
data-parallel, no collectives; host sums the 4 per-core partial projections
per batch (w_proj is row-parallel over head slices).
"""

import numpy as np

import concourse.bass as bass  # noqa: F401
import concourse.mybir as mybir
import concourse.tile as tile
from concourse import bacc
from concourse.bass_utils import run_bass_kernel_spmd

B, T, C, H, E, G = 2, 512, 256, 8, 32, 16
NCORES = 8
EP = E + 1      # 33: v block width per head (32 modulated v cols + ones col)
VW = 2 * EP     # 66: v block for both heads
TC = T // 128   # 4 row chunks

NG = 5          # bottom-NG g's per batch handled on GPSIMD
LAG = 3         # yT matmul emission lag (in batches) on the merged stream
FI = [512, 384, 256, 128]          # i-extent per j-chunk
PSTRIDE = [512, 512, 256, 128]     # psum plane stride per j-chunk
# per-jc PSUM batching: list of (g0, nplanes) with nplanes*PSTRIDE <= 1536
BATCHES = [
    [(0, 3), (3, 3), (6, 3), (9, 3), (12, 3), (15, 1)],
    [(0, 3), (3, 3), (6, 3), (9, 3), (12, 3), (15, 1)],
    [(0, 6), (6, 6), (12, 4)],
    [(0, 12), (12, 4)],
]

F16 = mybir.dt.float16
F32 = mybir.dt.float32
EXP = mybir.ActivationFunctionType.Exp

_NC_CACHE = [None]


def _build_nc():
    nc = bacc.Bacc("TRN2", target_bir_lowering=False)

    d_xT = nc.dram_tensor("xT", (C, T), F32, kind="ExternalInput")
    d_w = [
        nc.dram_tensor(f"wpl{jc}", (256, G * FI[jc]), F16, kind="ExternalInput")
        for jc in range(TC)
    ]
    d_wqk = nc.dram_tensor("w_qk", (C, 128), F32, kind="ExternalInput")
    d_wv = nc.dram_tensor("w_v", (C, 64), F32, kind="ExternalInput")
    d_ekT = nc.dram_tensor("ektabT", (64, G), F32, kind="ExternalInput")
    d_evb = nc.dram_tensor("evb", (128, G * VW), F16, kind="ExternalInput")
    d_wp = nc.dram_tensor("w_proj_sl", (64, C), F16, kind="ExternalInput")
    d_out = nc.dram_tensor("out", (T, C), F32, kind="ExternalOutput")

    with tile.TileContext(nc) as tc:
        with (
            tc.tile_pool(name="const", bufs=1) as const,
            tc.tile_pool(name="ps_misc", bufs=1, space="PSUM") as ps_misc,
            tc.tile_pool(name="ps_tri", bufs=2, space="PSUM") as ps_tri,
            tc.tile_pool(name="ps_y", bufs=1, space="PSUM") as ps_y,
            tc.tile_pool(name="pp", bufs=4) as pp,
            tc.tile_pool(name="e2p", bufs=6) as e2p,
        ):
            # ---------------- input loads ----------------
            xT_s = []
            for i in range(2):
                t_ = const.tile([128, T], F32, name=f"xT{i}")
                nc.sync.dma_start(out=t_[:], in_=d_xT[128 * i:128 * (i + 1), :])
                xT_s.append(t_)
            wqk_s = []
            for i in range(2):
                t_ = const.tile([128, 128], F32, name=f"wqk{i}")
                nc.sync.dma_start(out=t_[:], in_=d_wqk[128 * i:128 * (i + 1), :])
                wqk_s.append(t_)
            wv_s = []
            for i in range(2):
                t_ = const.tile([128, 64], F32, name=f"wv{i}")
                nc.sync.dma_start(out=t_[:], in_=d_wv[128 * i:128 * (i + 1), :])
                wv_s.append(t_)
            ekT_s = const.tile([64, G], F32, name="ekT")
            nc.sync.dma_start(out=ekT_s[:], in_=d_ekT[:, :])
            # masked-bias planes, in processing order (jc-major, h inner)
            w_s = [[None] * TC, [None] * TC]
            for jc in range(TC):
                for h in range(2):
                    t_ = const.tile([128, G * FI[jc]], F16, name=f"w{h}_{jc}")
                    nc.sync.dma_start(
                        out=t_[:], in_=d_w[jc][128 * h:128 * (h + 1), :])
                    w_s[h][jc] = t_
            evb_s = const.tile([128, G * VW], F16, name="evb")
            nc.sync.dma_start(out=evb_s[:], in_=d_evb[:, :])
            wp_s = const.tile([64, C], F16, name="wp")
            nc.sync.dma_start(out=wp_s[:], in_=d_wp[:, :])

            # ---------------- q/k/v projections ----------------
            # qk_ps rows: 0-31 qT(h0), 32-63 qT(h1), 64-95 kT(h0), 96-127 kT(h1)
            qk_ps = ps_misc.tile([128, T], F32, tag="misc", name="qk_ps")
            for ck in range(2):
                nc.tensor.matmul(
                    qk_ps[:],
                    lhsT=wqk_s[ck][:],
                    rhs=xT_s[ck][:],
                    start=(ck == 0), stop=(ck == 1),
                )
            qk_sb = const.tile([128, T], F16, name="qk_sb")
            nc.vector.tensor_copy(out=qk_sb[:], in_=qk_ps[:])

            # v_ps cols per t-chunk: 32 v(h0) | 32 v(h1)
            v_ps = ps_misc.tile([128, TC * 64], F32, tag="misc", name="v_ps")
            for tcc in range(TC):
                for ck in range(2):
                    nc.tensor.matmul(
                        v_ps[:, 64 * tcc:64 * (tcc + 1)],
                        lhsT=xT_s[ck][:, 128 * tcc:128 * (tcc + 1)],
                        rhs=wv_s[ck][:],
                        start=(ck == 0), stop=(ck == 1),
                    )

            # k tiles (contraction layout: e on partitions, j on free)
            k_h = []
            for h in range(2):
                t_ = const.tile([E, T], F16, name=f"k{h}")
                nc.vector.tensor_copy(out=t_[:], in_=qk_sb[64 + 32 * h:96 + 32 * h, :])
                k_h.append(t_)

            # q' tiles: per g block of T cols, rows = e, modulated by ektab/sqrt(E)
            q_all = []
            for h in range(2):
                t_ = const.tile([E, G * T], F16, name=f"qall{h}")
                q_all.append(t_)
            for g in range(G):
                for h in range(2):
                    nc.vector.tensor_scalar_mul(
                        q_all[h][:, T * g:T * (g + 1)],
                        qk_sb[32 * h:32 * h + 32, :],
                        ekT_s[32 * h:32 * h + 32, g:g + 1],
                    )

            # v66 per t-chunk: [v_h0(32) | 1 | v_h1(32) | 1]
            v66 = []
            for tcc in range(TC):
                t_ = const.tile([128, VW], F16, name=f"v66_{tcc}")
                nc.vector.tensor_copy(out=t_[:, 0:32], in_=v_ps[:, 64 * tcc:64 * tcc + 32])
                nc.vector.tensor_copy(out=t_[:, 33:65], in_=v_ps[:, 64 * tcc + 32:64 * tcc + 64])
                nc.vector.memset(t_[:, 32:33], 1.0)
                nc.vector.memset(t_[:, 65:66], 1.0)
                v66.append(t_)

            # v'_g = v66 * evtab broadcast: one wide TT per t-chunk
            vall = []
            for tcc in range(TC):
                t_ = const.tile([128, G * VW], F16, name=f"vall{tcc}")
                nc.vector.tensor_mul(
                    t_[:].rearrange("p (g w) -> p g w", g=G),
                    v66[tcc][:].unsqueeze(1).broadcast_to([128, G, VW]),
                    evb_s[:].rearrange("p (g w) -> p g w", g=G),
                )
                vall.append(t_)

            ones32 = const.tile([1, 32], F32, name="ones32")
            nc.vector.memset(ones32[:], 1.0)
            yTn = const.tile([64, T], F16, name="yTn")

            # ---------------- main attention loops ----------------
            # One shared PSUM bank holds both heads' accumulators:
            # h0 rows 0-32, h1 rows 64-96.
            yTT = ps_y.tile([64 + EP, T], F32, tag="y", name="yTT")
            first_h = [True, True]

            def emit_norm(h):
                recip = const.tile([1, T], F32, name=f"recip{h}")
                nc.vector.reciprocal(recip[:], yTT[64 * h + 32:64 * h + 33, :])
                bc_ps = ps_misc.tile([32, T], F32, tag="misc", name=f"bc{h}")
                nc.tensor.matmul(bc_ps[:], lhsT=ones32[:], rhs=recip[:],
                                 start=True, stop=True)
                rb_sb = const.tile([32, T], F32, name=f"rb{h}")
                nc.scalar.copy(rb_sb[:], bc_ps[:])
                nc.vector.tensor_mul(yTn[32 * h:32 * (h + 1), :],
                                     yTT[64 * h:64 * h + 32, :], rb_sb[:])

            def emit_yt(item):
                h, jc, g0, np_, last, e2b = item
                fi = FI[jc]
                s0 = 128 * jc
                for k in range(np_):
                    g = g0 + k
                    nc.tensor.matmul(
                        yTT[64 * h:64 * h + EP, s0:s0 + fi],
                        lhsT=vall[jc][:, VW * g + EP * h:VW * g + EP * (h + 1)],
                        rhs=e2b[:, k * fi:(k + 1) * fi],
                        start=first_h[h],
                        stop=(last and k == np_ - 1),
                    )
                    first_h[h] = False
                if last:
                    emit_norm(h)

            flat = []
            for jc in range(TC):
                for bi, (g0, np_) in enumerate(BATCHES[jc]):
                    for h in range(2):
                        flat.append((h, jc, g0, np_,
                                     bi == len(BATCHES[jc]) - 1 and jc == TC - 1))

            pending = []
            for (h, jc, g0, np_, last) in flat:
                s0 = 128 * jc
                fi = FI[jc]
                ps = PSTRIDE[jc]
                ghi = g0 + np_

                tri = ps_tri.tile([128, 1536], F32, tag="tri", name="tri")
                for k in range(np_):
                    g = g0 + k
                    nc.tensor.matmul(
                        tri[:, ps * k:ps * k + fi],
                        lhsT=k_h[h][:, s0:s0 + 128],
                        rhs=q_all[h][:, T * g + s0:T * (g + 1)],
                        start=True, stop=True,
                    )
                p_ = pp.tile([128, 1536], F16, tag="p", name="p")
                if ps == fi:
                    nc.scalar.activation(
                        p_[:, 0:np_ * fi], tri[:, 0:np_ * fi], EXP)
                else:
                    nc.scalar.activation(
                        p_[:, 0:np_ * ps].rearrange(
                            "p (k s) -> p k s", k=np_)[:, :, 0:fi],
                        tri[:, 0:np_ * ps].rearrange(
                            "p (k s) -> p k s", k=np_)[:, :, 0:fi],
                        EXP,
                    )

                # e2 = w * p.  First NG planes of the batch on gpsimd
                # (per-plane TTs), rest on vector (one wide strided TT).
                e2b = e2p.tile([128, 1536], F16, tag="e2", name="e2")
                nsplit = min(NG, np_)
                for k in range(nsplit):
                    g = g0 + k
                    nc.gpsimd.tensor_mul(
                        e2b[:, k * fi:(k + 1) * fi],
                        w_s[h][jc][:, g * fi:(g + 1) * fi],
                        p_[:, ps * k:ps * k + fi],
                    )
                if np_ > nsplit:
                    nrun = np_ - nsplit
                    psrc = p_[:, ps * nsplit:ps * np_].rearrange(
                        "p (k s) -> p k s", k=nrun)[:, :, 0:fi]
                    nc.vector.tensor_mul(
                        e2b[:, nsplit * fi:np_ * fi].rearrange(
                            "p (k s) -> p k s", k=nrun),
                        w_s[h][jc][:, (g0 + nsplit) * fi:ghi * fi].rearrange(
                            "p (k s) -> p k s", k=nrun),
                        psrc,
                    )

                pending.append((h, jc, g0, np_, last, e2b))
                if len(pending) > LAG:
                    emit_yt(pending.pop(0))
            for item in pending:
                emit_yt(item)

            # ---------------- output projection ----------------
            for tcc in range(TC):
                o_ps = ps_misc.tile([128, C], F32, tag="misc", name=f"o_ps{tcc}")
                nc.tensor.matmul(
                    o_ps[:],
                    lhsT=yTn[:, 128 * tcc:128 * (tcc + 1)],
                    rhs=wp_s[:],
                    start=True, stop=True,
                )
                o_sb = const.tile([128, C], F32, name=f"o_sb{tcc}")
                nc.scalar.copy(o_sb[:], o_ps[:])
                nc.sync.dma_start(out=d_out[128 * tcc:128 * (tcc + 1), :], in_=o_sb[:])

    nc.compile()
    return nc


def _get_nc():
    if _NC_CACHE[0] is None:
        _NC_CACHE[0] = _build_nc()
    return _NC_CACHE[0]


def _prep_core_inputs(c, x, bm, w_attn, w_proj, w_edge_k, w_edge_v, eet, abt):
    b, hp = divmod(c, 4)
    h0 = 2 * hp
    xT = np.ascontiguousarray(x[b].T).astype(np.float32)            # (C, T)
    triu = np.triu(np.ones((T, T), dtype=bool))                     # j <= i
    bmT = bm[b].T                                                   # bmT[j,i] = bm[b][i,j]
    mm = np.where(triu, bm[b], 255)                                 # (T, T) [j,i]
    ebias = np.exp(abt)                                             # (G, H)
    # masked-bias planes: w[h][j, g, i] = (mm[j,i]==g) * exp(bias[bm[i,j], h])
    wpl = []
    for jc in range(TC):
        s0, fi = 128 * jc, FI[jc]
        wp_ = np.zeros((2, 128, G, fi), np.float16)
        sub = mm[s0:s0 + 128, s0:s0 + fi]
        for h in range(2):
            cbsub = ebias[:, h0 + h][bmT[s0:s0 + 128, s0:s0 + fi]]
            for g in range(G):
                wp_[h, :, g, :] = (sub == g) * cbsub
        wpl.append(np.ascontiguousarray(wp_.reshape(256, G * fi)))
    w_qk = np.concatenate(
        [w_attn[:, hp * 64:(hp + 1) * 64],
         w_attn[:, C + hp * 64:C + (hp + 1) * 64]], axis=1
    ).astype(np.float32)                                            # (C, 128)
    w_v = np.ascontiguousarray(
        w_attn[:, 2 * C + hp * 64:2 * C + (hp + 1) * 64]).astype(np.float32)
    ektab = (eet @ w_edge_k)[:, hp * 64:(hp + 1) * 64] / np.sqrt(E)  # (G, 64)
    ektabT = np.ascontiguousarray(ektab.T).astype(np.float32)        # (64, G)
    evtab = (eet @ w_edge_v)[:, hp * 64:(hp + 1) * 64]               # (G, 64)
    evb = np.zeros((128, G * VW), np.float16)
    for g in range(G):
        evb[:, VW * g:VW * g + 32] = evtab[g, 0:32].astype(np.float16)[None, :]
        evb[:, VW * g + 32] = 1.0
        evb[:, VW * g + 33:VW * g + 65] = evtab[g, 32:64].astype(np.float16)[None, :]
        evb[:, VW * g + 65] = 1.0
    w_proj_sl = np.ascontiguousarray(
        w_proj[hp * 64:(hp + 1) * 64, :]).astype(np.float16)         # (64, C)
    d = {
        "xT": xT, "w_qk": w_qk, "w_v": w_v,
        "ektabT": ektabT, "evb": evb, "w_proj_sl": w_proj_sl,
    }
    for jc in range(TC):
        d[f"wpl{jc}"] = wpl[jc]
    return d


def run(inputs, trace=False):
    x = np.asarray(inputs["x"], np.float32)
    bm = np.asarray(inputs["bias_matrix"]).astype(np.int64)
    w_attn = np.asarray(inputs["w_attn"], np.float32)
    w_proj = np.asarray(inputs["w_proj"], np.float32)
    w_edge_k = np.asarray(inputs["w_edge_k"], np.float32)
    w_edge_v = np.asarray(inputs["w_edge_v"], np.float32)
    eet = np.asarray(inputs["edge_emb_table"], np.float32)
    abt = np.asarray(inputs["attn_bias_table"], np.float32)

    nc = _get_nc()
    in_maps = [
        _prep_core_inputs(c, x, bm, w_attn, w_proj, w_edge_k, w_edge_v, eet, abt)
        for c in range(NCORES)
    ]
    res = run_bass_kernel_spmd(nc, in_maps, core_ids=list(range(NCORES)),
                               trace=trace)
    out = np.zeros((B, T, C), np.float32)
    for c in range(NCORES):
        out[c // 4] += res.results[c]["out"]
    return out, res


def kernel(**inputs) -> np.ndarray:
    out, _ = run(inputs, trace=False)
    return out


# revision 15
# speedup vs baseline: 1.1391x; 1.1391x over previous
"""Edge-conditioned causal self-attention (Graphormer-style) on 8 Trainium2 cores.

Math (per batch b, head h; i = query pos, j = key pos; E=32, G=16 edge types):
  scores[i,j] = sum_e q[i,e] * k[j,e] * ektab[bm[j,i], e] / sqrt(E)
  attn        = softmax_j(scores + biastab[bm[i,j]])  (causal j<=i)
  y[i,e]      = sum_j attn[i,j] * v[j,e] * evtab[bm[j,i], e]
  out         = y @ w_proj
where ektab = edge_emb @ w_edge_k (per-head slice), evtab likewise, and bm is
the integer edge-type matrix.  NOTE: the additive bias indexes bm TRANSPOSED
relative to the k/v modulation (matches the reference einsums).

Since there are only G=16 edge types, scores decompose into 16 per-type Gram
planes S_g = (q .* ektab[g]) @ k^T.  The per-(j,i) selection and the additive
bias are both folded into host-precomputed f16 planes
  w_g[j,i] = (bm[j,i]==g & j<=i) * exp(biastab[bm[i,j]])
so the masked-biased softmax numerators are e2_g = w_g * exp(S_g), and the
output accumulates yT += (v .* evtab[g] | 1) @ e2_g with the ones column
giving the softmax denominator for free.

Schedule (engine-balanced for the TimelineSim cost model):
  - the h0/h1 head streams are interleaved batch-by-batch (shared one-bank
    yT PSUM tile: h0 rows 0-32, h1 rows 64-96) so the PE always has
    independent ready matmuls and stays at full clock;
  - S_g matmuls grouped into 3-bank PSUM tiles, exp'd in one batched strided
    activation (Scalar is the only engine with a cheap PSUM exit);
  - e2 multiplies split: bottom NG g's per batch on GPSIMD (per-plane TTs),
    rest on Vector (one wide strided TT per batch, 2x rate);
  - yT matmuls are emitted LAG batches late so the in-order PE sequencer
    never head-of-line blocks on a dependency-stalled yT matmul.
Sharding: core c -> batch b=c//4, head pair (2*(c%4), 2*(c%4)+1).  Fully
data-parallel, no collectives; host sums the 4 per-core partial projections
per batch (w_proj is row-parallel over head slices).
"""

import numpy as np

import concourse.bass as bass  # noqa: F401
import concourse.mybir as mybir
import concourse.tile as tile
from concourse import bacc
from concourse.bass_utils import run_bass_kernel_spmd

B, T, C, H, E, G = 2, 512, 256, 8, 32, 16
NCORES = 8
EP = E + 1      # 33: v block width per head (32 modulated v cols + ones col)
VW = 2 * EP     # 66: v block for both heads
TC = T // 128   # 4 row chunks

NG = 5          # bottom-NG g's per batch handled on GPSIMD
LAG = 3         # yT matmul emission lag (in batches) on the merged stream
FI = [512, 384, 256, 128]          # i-extent per j-chunk
PSTRIDE = [512, 512, 256, 128]     # psum plane stride per j-chunk
# per-jc PSUM batching: list of (g0, nplanes) with nplanes*PSTRIDE <= 1536
BATCHES = [
    [(0, 3), (3, 3), (6, 3), (9, 3), (12, 3), (15, 1)],
    [(0, 3), (3, 3), (6, 3), (9, 3), (12, 3), (15, 1)],
    [(0, 6), (6, 6), (12, 4)],
    [(0, 12), (12, 4)],
]

F16 = mybir.dt.float16
F32 = mybir.dt.float32
EXP = mybir.ActivationFunctionType.Exp

_NC_CACHE = [None]


def _build_nc():
    nc = bacc.Bacc("TRN2", target_bir_lowering=False)

    d_xT = nc.dram_tensor("xT", (C, T), F32, kind="ExternalInput")
    d_w = [
        nc.dram_tensor(f"wpl{jc}", (256, G * FI[jc]), F16, kind="ExternalInput")
        for jc in range(TC)
    ]
    d_wqk = nc.dram_tensor("w_qk", (C, 128), F32, kind="ExternalInput")
    d_wv = nc.dram_tensor("w_v", (C, 64), F32, kind="ExternalInput")
    d_ekT = nc.dram_tensor("ektabT", (64, G), F32, kind="ExternalInput")
    d_evb = nc.dram_tensor("evb", (128, G * VW), F16, kind="ExternalInput")
    d_wp = nc.dram_tensor("w_proj_sl", (64, C), F16, kind="ExternalInput")
    d_out = nc.dram_tensor("out", (T, C), F32, kind="ExternalOutput")

    with tile.TileContext(nc) as tc:
        with (
            tc.tile_pool(name="const", bufs=1) as const,
            tc.tile_pool(name="ps_misc", bufs=1, space="PSUM") as ps_misc,
            tc.tile_pool(name="ps_tri", bufs=2, space="PSUM") as ps_tri,
            tc.tile_pool(name="ps_y", bufs=1, space="PSUM") as ps_y,
            tc.tile_pool(name="pp", bufs=4) as pp,
            tc.tile_pool(name="e2p", bufs=6) as e2p,
        ):
            # ---------------- input loads ----------------
            xT_s = []
            for i in range(2):
                t_ = const.tile([128, T], F32, name=f"xT{i}")
                nc.sync.dma_start(out=t_[:], in_=d_xT[128 * i:128 * (i + 1), :])
                xT_s.append(t_)
            wqk_s = []
            for i in range(2):
                t_ = const.tile([128, 128], F32, name=f"wqk{i}")
                nc.sync.dma_start(out=t_[:], in_=d_wqk[128 * i:128 * (i + 1), :])
                wqk_s.append(t_)
            wv_s = []
            for i in range(2):
                t_ = const.tile([128, 64], F32, name=f"wv{i}")
                nc.sync.dma_start(out=t_[:], in_=d_wv[128 * i:128 * (i + 1), :])
                wv_s.append(t_)
            ekT_s = const.tile([64, G], F32, name="ekT")
            nc.sync.dma_start(out=ekT_s[:], in_=d_ekT[:, :])
            # masked-bias planes, in processing order (jc-major, h inner)
            w_s = [[None] * TC, [None] * TC]
            for jc in range(TC):
                for h in range(2):
                    t_ = const.tile([128, G * FI[jc]], F16, name=f"w{h}_{jc}")
                    nc.sync.dma_start(
                        out=t_[:], in_=d_w[jc][128 * h:128 * (h + 1), :])
                    w_s[h][jc] = t_
            evb_s = const.tile([128, G * VW], F16, name="evb")
            nc.sync.dma_start(out=evb_s[:], in_=d_evb[:, :])
            wp_s = const.tile([64, C], F16, name="wp")
            nc.sync.dma_start(out=wp_s[:], in_=d_wp[:, :])

            # ---------------- q/k/v projections ----------------
            # qk_ps rows: 0-31 qT(h0), 32-63 qT(h1), 64-95 kT(h0), 96-127 kT(h1)
            qk_ps = ps_misc.tile([128, T], F32, tag="misc", name="qk_ps")
            for ck in range(2):
                nc.tensor.matmul(
                    qk_ps[:],
                    lhsT=wqk_s[ck][:],
                    rhs=xT_s[ck][:],
                    start=(ck == 0), stop=(ck == 1),
                )
            qk_sb = const.tile([128, T], F16, name="qk_sb")
            nc.vector.tensor_copy(out=qk_sb[:], in_=qk_ps[:])

            # v_ps cols per t-chunk: 32 v(h0) | 32 v(h1)
            v_ps = ps_misc.tile([128, TC * 64], F32, tag="misc", name="v_ps")
            for tcc in range(TC):
                for ck in range(2):
                    nc.tensor.matmul(
                        v_ps[:, 64 * tcc:64 * (tcc + 1)],
                        lhsT=xT_s[ck][:, 128 * tcc:128 * (tcc + 1)],
                        rhs=wv_s[ck][:],
                        start=(ck == 0), stop=(ck == 1),
                    )

            # k tiles (contraction layout: e on partitions, j on free)
            k_h = []
            for h in range(2):
                t_ = const.tile([E, T], F16, name=f"k{h}")
                nc.vector.tensor_copy(out=t_[:], in_=qk_sb[64 + 32 * h:96 + 32 * h, :])
                k_h.append(t_)

            # q' tiles: per g block of T cols, rows = e, modulated by ektab/sqrt(E)
            q_all = []
            for h in range(2):
                t_ = const.tile([E, G * T], F16, name=f"qall{h}")
                q_all.append(t_)
            for g in range(G):
                for h in range(2):
                    nc.vector.tensor_scalar_mul(
                        q_all[h][:, T * g:T * (g + 1)],
                        qk_sb[32 * h:32 * h + 32, :],
                        ekT_s[32 * h:32 * h + 32, g:g + 1],
                    )

            # v66 per t-chunk: [v_h0(32) | 1 | v_h1(32) | 1]
            v66 = []
            for tcc in range(TC):
                t_ = const.tile([128, VW], F16, name=f"v66_{tcc}")
                nc.vector.tensor_copy(out=t_[:, 0:32], in_=v_ps[:, 64 * tcc:64 * tcc + 32])
                nc.vector.tensor_copy(out=t_[:, 33:65], in_=v_ps[:, 64 * tcc + 32:64 * tcc + 64])
                nc.vector.memset(t_[:, 32:33], 1.0)
                nc.vector.memset(t_[:, 65:66], 1.0)
                v66.append(t_)

            # v'_g = v66 * evtab broadcast: one wide TT per t-chunk
            vall = []
            for tcc in range(TC):
                t_ = const.tile([128, G * VW], F16, name=f"vall{tcc}")
                nc.vector.tensor_mul(
                    t_[:].rearrange("p (g w) -> p g w", g=G),
                    v66[tcc][:].unsqueeze(1).broadcast_to([128, G, VW]),
                    evb_s[:].rearrange("p (g w) -> p g w", g=G),
                )
                vall.append(t_)

            ones32 = const.tile([1, 32], F32, name="ones32")
            nc.vector.memset(ones32[:], 1.0)
            yTn = const.tile([64, T], F16, name="yTn")

            # ---------------- main attention loops ----------------
            # One shared PSUM bank holds both heads' accumulators:
            # h0 rows 0-32, h1 rows 64-96.
            yTT = ps_y.tile([64 + EP, T], F32, tag="y", name="yTT")
            first_h = [True, True]

            def emit_norm(h):
                recip = const.tile([1, T], F32, name=f"recip{h}")
                nc.vector.reciprocal(recip[:], yTT[64 * h + 32:64 * h + 33, :])
                bc_ps = ps_misc.tile([32, T], F32, tag="misc", name=f"bc{h}")
                nc.tensor.matmul(bc_ps[:], lhsT=ones32[:], rhs=recip[:],
                                 start=True, stop=True)
                rb_sb = const.tile([32, T], F32, name=f"rb{h}")
                nc.scalar.copy(rb_sb[:], bc_ps[:])
                nc.vector.tensor_mul(yTn[32 * h:32 * (h + 1), :],
                                     yTT[64 * h:64 * h + 32, :], rb_sb[:])

            def emit_yt(item):
                h, jc, g0, np_, last, e2b = item
                fi = FI[jc]
                s0 = 128 * jc
                for k in range(np_):
                    g = g0 + k
                    nc.tensor.matmul(
                        yTT[64 * h:64 * h + EP, s0:s0 + fi],
                        lhsT=vall[jc][:, VW * g + EP * h:VW * g + EP * (h + 1)],
                        rhs=e2b[:, k * fi:(k + 1) * fi],
                        start=first_h[h],
                        stop=(last and k == np_ - 1),
                    )
                    first_h[h] = False
                if last:
                    emit_norm(h)

            flat = []
            for jc in range(TC):
                for bi, (g0, np_) in enumerate(BATCHES[jc]):
                    for h in range(2):
                        flat.append((h, jc, g0, np_,
                                     bi == len(BATCHES[jc]) - 1 and jc == TC - 1))

            pending = []
            for (h, jc, g0, np_, last) in flat:
                s0 = 128 * jc
                fi = FI[jc]
                ps = PSTRIDE[jc]
                ghi = g0 + np_

                tri = ps_tri.tile([128, 1536], F32, tag="tri", name="tri")
                for k in range(np_):
                    g = g0 + k
                    nc.tensor.matmul(
                        tri[:, ps * k:ps * k + fi],
                        lhsT=k_h[h][:, s0:s0 + 128],
                        rhs=q_all[h][:, T * g + s0:T * (g + 1)],
                        start=True, stop=True,
                    )
                p_ = pp.tile([128, 1536], F16, tag="p", name="p")
                if ps == fi:
                    nc.scalar.activation(
                        p_[:, 0:np_ * fi], tri[:, 0:np_ * fi], EXP)
                else:
                    nc.scalar.activation(
                        p_[:, 0:np_ * ps].rearrange(
                            "p (k s) -> p k s", k=np_)[:, :, 0:fi],
                        tri[:, 0:np_ * ps].rearrange(
                            "p (k s) -> p k s", k=np_)[:, :, 0:fi],
                        EXP,
                    )

                # e2 = w * p.  Planes with g < NG on gpsimd (per-plane TTs),
                # rest on vector (one wide strided TT).
                e2b = e2p.tile([128, 1536], F16, tag="e2", name="e2")
                nsplit = max(0, min(NG - g0, np_))
                for k in range(nsplit):
                    g = g0 + k
                    nc.gpsimd.tensor_mul(
                        e2b[:, k * fi:(k + 1) * fi],
                        w_s[h][jc][:, g * fi:(g + 1) * fi],
                        p_[:, ps * k:ps * k + fi],
                    )
                if np_ > nsplit:
                    nrun = np_ - nsplit
                    psrc = p_[:, ps * nsplit:ps * np_].rearrange(
                        "p (k s) -> p k s", k=nrun)[:, :, 0:fi]
                    nc.vector.tensor_mul(
                        e2b[:, nsplit * fi:np_ * fi].rearrange(
                            "p (k s) -> p k s", k=nrun),
                        w_s[h][jc][:, (g0 + nsplit) * fi:ghi * fi].rearrange(
                            "p (k s) -> p k s", k=nrun),
                        psrc,
                    )

                pending.append((h, jc, g0, np_, last, e2b))
                if len(pending) > LAG:
                    emit_yt(pending.pop(0))
            for item in pending:
                emit_yt(item)

            # ---------------- output projection ----------------
            for tcc in range(TC):
                o_ps = ps_misc.tile([128, C], F32, tag="misc", name=f"o_ps{tcc}")
                nc.tensor.matmul(
                    o_ps[:],
                    lhsT=yTn[:, 128 * tcc:128 * (tcc + 1)],
                    rhs=wp_s[:],
                    start=True, stop=True,
                )
                o_sb = const.tile([128, C], F32, name=f"o_sb{tcc}")
                nc.scalar.copy(o_sb[:], o_ps[:])
                nc.sync.dma_start(out=d_out[128 * tcc:128 * (tcc + 1), :], in_=o_sb[:])

    nc.compile()
    return nc


def _get_nc():
    if _NC_CACHE[0] is None:
        _NC_CACHE[0] = _build_nc()
    return _NC_CACHE[0]


def _prep_core_inputs(c, x, bm, w_attn, w_proj, w_edge_k, w_edge_v, eet, abt):
    b, hp = divmod(c, 4)
    h0 = 2 * hp
    xT = np.ascontiguousarray(x[b].T).astype(np.float32)            # (C, T)
    triu = np.triu(np.ones((T, T), dtype=bool))                     # j <= i
    bmT = bm[b].T                                                   # bmT[j,i] = bm[b][i,j]
    mm = np.where(triu, bm[b], 255)                                 # (T, T) [j,i]
    ebias = np.exp(abt)                                             # (G, H)
    # masked-bias planes: w[h][j, g, i] = (mm[j,i]==g) * exp(bias[bm[i,j], h])
    wpl = []
    for jc in range(TC):
        s0, fi = 128 * jc, FI[jc]
        wp_ = np.zeros((2, 128, G, fi), np.float16)
        sub = mm[s0:s0 + 128, s0:s0 + fi]
        for h in range(2):
            cbsub = ebias[:, h0 + h][bmT[s0:s0 + 128, s0:s0 + fi]]
            for g in range(G):
                wp_[h, :, g, :] = (sub == g) * cbsub
        wpl.append(np.ascontiguousarray(wp_.reshape(256, G * fi)))
    w_qk = np.concatenate(
        [w_attn[:, hp * 64:(hp + 1) * 64],
         w_attn[:, C + hp * 64:C + (hp + 1) * 64]], axis=1
    ).astype(np.float32)                                            # (C, 128)
    w_v = np.ascontiguousarray(
        w_attn[:, 2 * C + hp * 64:2 * C + (hp + 1) * 64]).astype(np.float32)
    ektab = (eet @ w_edge_k)[:, hp * 64:(hp + 1) * 64] / np.sqrt(E)  # (G, 64)
    ektabT = np.ascontiguousarray(ektab.T).astype(np.float32)        # (64, G)
    evtab = (eet @ w_edge_v)[:, hp * 64:(hp + 1) * 64]               # (G, 64)
    evb = np.zeros((128, G * VW), np.float16)
    for g in range(G):
        evb[:, VW * g:VW * g + 32] = evtab[g, 0:32].astype(np.float16)[None, :]
        evb[:, VW * g + 32] = 1.0
        evb[:, VW * g + 33:VW * g + 65] = evtab[g, 32:64].astype(np.float16)[None, :]
        evb[:, VW * g + 65] = 1.0
    w_proj_sl = np.ascontiguousarray(
        w_proj[hp * 64:(hp + 1) * 64, :]).astype(np.float16)         # (64, C)
    d = {
        "xT": xT, "w_qk": w_qk, "w_v": w_v,
        "ektabT": ektabT, "evb": evb, "w_proj_sl": w_proj_sl,
    }
    for jc in range(TC):
        d[f"wpl{jc}"] = wpl[jc]
    return d


def run(inputs, trace=False):
    x = np.asarray(inputs["x"], np.float32)
    bm = np.asarray(inputs["bias_matrix"]).astype(np.int64)
    w_attn = np.asarray(inputs["w_attn"], np.float32)
    w_proj = np.asarray(inputs["w_proj"], np.float32)
    w_edge_k = np.asarray(inputs["w_edge_k"], np.float32)
    w_edge_v = np.asarray(inputs["w_edge_v"], np.float32)
    eet = np.asarray(inputs["edge_emb_table"], np.float32)
    abt = np.asarray(inputs["attn_bias_table"], np.float32)

    nc = _get_nc()
    in_maps = [
        _prep_core_inputs(c, x, bm, w_attn, w_proj, w_edge_k, w_edge_v, eet, abt)
        for c in range(NCORES)
    ]
    res = run_bass_kernel_spmd(nc, in_maps, core_ids=list(range(NCORES)),
                               trace=trace)
    out = np.zeros((B, T, C), np.float32)
    for c in range(NCORES):
        out[c // 4] += res.results[c]["out"]
    return out, res


def kernel(**inputs) -> np.ndarray:
    out, _ = run(inputs, trace=False)
    return out


# revision 16
# speedup vs baseline: 1.4612x; 1.2827x over previous
"""Edge-conditioned causal self-attention (Graphormer-style) on 8 Trainium2 cores.

Math (per batch b, head h; i = query pos, j = key pos; E=32, G=16 edge types):
  scores[i,j] = sum_e q[i,e] * k[j,e] * ektab[bm[j,i], e] / sqrt(E)
  attn        = softmax_j(scores + biastab[bm[i,j]])  (causal j<=i)
  y[i,e]      = sum_j attn[i,j] * v[j,e] * evtab[bm[j,i], e]
  out         = y @ w_proj
where ektab = edge_emb @ w_edge_k (per-head slice), evtab likewise, and bm is
the integer edge-type matrix.  NOTE: the additive bias indexes bm TRANSPOSED
relative to the k/v modulation (matches the reference einsums).

Since there are only G=16 edge types, scores decompose into 16 per-type Gram
planes S_g = (q .* ektab[g]) @ k^T.  The per-(j,i) selection and the additive
bias are both folded into host-precomputed f16 planes
  w_g[j,i] = (bm[j,i]==g & j<=i) * exp(biastab[bm[i,j]])
so the masked-biased softmax numerators are e2_g = w_g * exp(S_g), and the
output accumulates yT += (v .* evtab[g] | 1) @ e2_g with the ones column
giving the softmax denominator for free.

Schedule (engine-balanced for the TimelineSim cost model):
  - the h0/h1 head streams are interleaved batch-by-batch (shared one-bank
    yT PSUM tile: h0 rows 0-32, h1 rows 64-96) so the PE always has
    independent ready matmuls and stays at full clock;
  - S_g matmuls grouped into 3-bank PSUM tiles, exp'd in one batched strided
    activation (Scalar is the only engine with a cheap PSUM exit);
  - e2 multiplies split: bottom NG g's per batch on GPSIMD (per-plane TTs),
    rest on Vector (one wide strided TT per batch, 2x rate);
  - yT matmuls are emitted LAG batches late so the in-order PE sequencer
    never head-of-line blocks on a dependency-stalled yT matmul.
Sharding: core c -> batch b=c//4, head pair (2*(c%4), 2*(c%4)+1).  Fully
data-parallel, no collectives; host sums the 4 per-core partial projections
per batch (w_proj is row-parallel over head slices).
"""

import numpy as np

import concourse.bass as bass  # noqa: F401
import concourse.mybir as mybir
import concourse.tile as tile
from concourse import bacc
from concourse.bass_utils import run_bass_kernel_spmd

B, T, C, H, E, G = 2, 512, 256, 8, 32, 16
NCORES = 8
EP = E + 1      # 33: v block width per head (32 modulated v cols + ones col)
VW = 2 * EP     # 66: v block for both heads
TC = T // 128   # 4 row chunks

NG = 5          # bottom-NG g's per batch handled on GPSIMD
LAG = 3         # yT matmul emission lag (in batches) on the merged stream
FI = [512, 384, 256, 128]          # i-extent per j-chunk
PSTRIDE = [512, 512, 256, 128]     # psum plane stride per j-chunk
# per-jc PSUM batching: list of (g0, nplanes) with nplanes*PSTRIDE <= 1536
BATCHES = [
    [(0, 3), (3, 3), (6, 3), (9, 3), (12, 3), (15, 1)],
    [(0, 3), (3, 3), (6, 3), (9, 3), (12, 3), (15, 1)],
    [(0, 6), (6, 6), (12, 4)],
    [(0, 12), (12, 4)],
]

F16 = mybir.dt.float16
F32 = mybir.dt.float32
EXP = mybir.ActivationFunctionType.Exp

_NC_CACHE = [None]


def _build_nc():
    nc = bacc.Bacc("TRN2", target_bir_lowering=False)

    d_xT = nc.dram_tensor("xT", (C, T), F32, kind="ExternalInput")
    d_w = [
        nc.dram_tensor(f"wpl{jc}", (256, G * FI[jc]), F16, kind="ExternalInput")
        for jc in range(TC)
    ]
    d_wqk = nc.dram_tensor("w_qk", (C, 128), F32, kind="ExternalInput")
    d_wv = nc.dram_tensor("w_v", (C, 64), F32, kind="ExternalInput")
    d_ekT = nc.dram_tensor("ektabT", (64, G), F32, kind="ExternalInput")
    d_evb = nc.dram_tensor("evb", (128, G * VW), F16, kind="ExternalInput")
    d_wp = nc.dram_tensor("w_proj_sl", (64, C), F16, kind="ExternalInput")
    d_out = nc.dram_tensor("out", (T, C), F32, kind="ExternalOutput")

    with tile.TileContext(nc) as tc:
        with (
            tc.tile_pool(name="const", bufs=1) as const,
            tc.tile_pool(name="ps_misc", bufs=1, space="PSUM") as ps_misc,
            tc.tile_pool(name="ps_tri", bufs=2, space="PSUM") as ps_tri,
            tc.tile_pool(name="ps_y", bufs=1, space="PSUM") as ps_y,
            tc.tile_pool(name="pp", bufs=4) as pp,
            tc.tile_pool(name="e2p", bufs=6) as e2p,
        ):
            # ---------------- input loads ----------------
            xT_s = []
            for i in range(2):
                t_ = const.tile([128, T], F32, name=f"xT{i}")
                nc.sync.dma_start(out=t_[:], in_=d_xT[128 * i:128 * (i + 1), :])
                xT_s.append(t_)
            wqk_s = []
            for i in range(2):
                t_ = const.tile([128, 128], F32, name=f"wqk{i}")
                nc.sync.dma_start(out=t_[:], in_=d_wqk[128 * i:128 * (i + 1), :])
                wqk_s.append(t_)
            wv_s = []
            for i in range(2):
                t_ = const.tile([128, 64], F32, name=f"wv{i}")
                nc.sync.dma_start(out=t_[:], in_=d_wv[128 * i:128 * (i + 1), :])
                wv_s.append(t_)
            ekT_s = const.tile([64, G], F32, name="ekT")
            nc.sync.dma_start(out=ekT_s[:], in_=d_ekT[:, :])
            evb_s = const.tile([128, G * VW], F16, name="evb")
            nc.sync.dma_start(out=evb_s[:], in_=d_evb[:, :])
            wp_s = const.tile([64, C], F16, name="wp")
            nc.sync.dma_start(out=wp_s[:], in_=d_wp[:, :])
            # masked-bias planes, in processing order (jc-major, h inner),
            # two half-tile DMAs each so early batches unblock sooner
            w_s = [[None] * TC, [None] * TC]
            for jc in range(TC):
                half = G * FI[jc] // 2
                for h in range(2):
                    t_ = const.tile([128, G * FI[jc]], F16, name=f"w{h}_{jc}")
                    nc.sync.dma_start(
                        out=t_[:, 0:half],
                        in_=d_w[jc][128 * h:128 * (h + 1), 0:half])
                    nc.sync.dma_start(
                        out=t_[:, half:],
                        in_=d_w[jc][128 * h:128 * (h + 1), half:])
                    w_s[h][jc] = t_

            # ---------------- q/k/v projections ----------------
            # qk_ps rows: 0-31 qT(h0), 32-63 qT(h1), 64-95 kT(h0), 96-127 kT(h1)
            qk_ps = ps_misc.tile([128, T], F32, tag="misc", name="qk_ps")
            for ck in range(2):
                nc.tensor.matmul(
                    qk_ps[:],
                    lhsT=wqk_s[ck][:],
                    rhs=xT_s[ck][:],
                    start=(ck == 0), stop=(ck == 1),
                )
            qk_sb = const.tile([128, T], F16, name="qk_sb")
            nc.vector.tensor_copy(out=qk_sb[:], in_=qk_ps[:])

            # v_ps cols per t-chunk: 32 v(h0) | 32 v(h1)
            v_ps = ps_misc.tile([128, TC * 64], F32, tag="misc", name="v_ps")
            for tcc in range(TC):
                for ck in range(2):
                    nc.tensor.matmul(
                        v_ps[:, 64 * tcc:64 * (tcc + 1)],
                        lhsT=xT_s[ck][:, 128 * tcc:128 * (tcc + 1)],
                        rhs=wv_s[ck][:],
                        start=(ck == 0), stop=(ck == 1),
                    )

            # k tiles (contraction layout: e on partitions, j on free)
            k_h = []
            for h in range(2):
                t_ = const.tile([E, T], F16, name=f"k{h}")
                nc.vector.tensor_copy(out=t_[:], in_=qk_sb[64 + 32 * h:96 + 32 * h, :])
                k_h.append(t_)

            # q' tiles: per g block of T cols, rows = e, modulated by ektab/sqrt(E)
            q_all = []
            for h in range(2):
                t_ = const.tile([E, G * T], F16, name=f"qall{h}")
                q_all.append(t_)
            for g in range(G):
                for h in range(2):
                    nc.vector.tensor_scalar_mul(
                        q_all[h][:, T * g:T * (g + 1)],
                        qk_sb[32 * h:32 * h + 32, :],
                        ekT_s[32 * h:32 * h + 32, g:g + 1],
                    )

            # v66 per t-chunk: [v_h0(32) | 1 | v_h1(32) | 1]
            v66 = []
            for tcc in range(TC):
                t_ = const.tile([128, VW], F16, name=f"v66_{tcc}")
                nc.vector.tensor_copy(out=t_[:, 0:32], in_=v_ps[:, 64 * tcc:64 * tcc + 32])
                nc.vector.tensor_copy(out=t_[:, 33:65], in_=v_ps[:, 64 * tcc + 32:64 * tcc + 64])
                nc.vector.memset(t_[:, 32:33], 1.0)
                nc.vector.memset(t_[:, 65:66], 1.0)
                v66.append(t_)

            # v'_g = v66 * evtab broadcast: one wide TT per t-chunk
            vall = []
            for tcc in range(TC):
                t_ = const.tile([128, G * VW], F16, name=f"vall{tcc}")
                nc.vector.tensor_mul(
                    t_[:].rearrange("p (g w) -> p g w", g=G),
                    v66[tcc][:].unsqueeze(1).broadcast_to([128, G, VW]),
                    evb_s[:].rearrange("p (g w) -> p g w", g=G),
                )
                vall.append(t_)

            ones32 = const.tile([1, 32], F32, name="ones32")
            nc.vector.memset(ones32[:], 1.0)
            yTn = const.tile([64, T], F16, name="yTn")

            # ---------------- main attention loops ----------------
            # One shared PSUM bank holds both heads' accumulators:
            # h0 rows 0-32, h1 rows 64-96.
            yTT = ps_y.tile([64 + EP, T], F32, tag="y", name="yTT")
            first_h = [True, True]

            def emit_norm(h):
                recip = const.tile([1, T], F32, name=f"recip{h}")
                nc.vector.reciprocal(recip[:], yTT[64 * h + 32:64 * h + 33, :])
                bc_ps = ps_misc.tile([32, T], F32, tag="misc", name=f"bc{h}")
                nc.tensor.matmul(bc_ps[:], lhsT=ones32[:], rhs=recip[:],
                                 start=True, stop=True)
                rb_sb = const.tile([32, T], F32, name=f"rb{h}")
                nc.scalar.copy(rb_sb[:], bc_ps[:])
                nc.vector.tensor_mul(yTn[32 * h:32 * (h + 1), :],
                                     yTT[64 * h:64 * h + 32, :], rb_sb[:])

            def emit_yt(item):
                h, jc, g0, np_, last, e2b = item
                fi = FI[jc]
                s0 = 128 * jc
                for k in range(np_):
                    g = g0 + k
                    nc.tensor.matmul(
                        yTT[64 * h:64 * h + EP, s0:s0 + fi],
                        lhsT=vall[jc][:, VW * g + EP * h:VW * g + EP * (h + 1)],
                        rhs=e2b[:, k * fi:(k + 1) * fi],
                        start=first_h[h],
                        stop=(last and k == np_ - 1),
                    )
                    first_h[h] = False
                if last:
                    emit_norm(h)

            flat = []
            for jc in range(TC):
                for bi, (g0, np_) in enumerate(BATCHES[jc]):
                    for h in range(2):
                        flat.append((h, jc, g0, np_,
                                     bi == len(BATCHES[jc]) - 1 and jc == TC - 1))

            pending = []
            for (h, jc, g0, np_, last) in flat:
                s0 = 128 * jc
                fi = FI[jc]
                ps = PSTRIDE[jc]
                ghi = g0 + np_

                tri = ps_tri.tile([128, 1536], F32, tag="tri", name="tri")
                for k in range(np_):
                    g = g0 + k
                    nc.tensor.matmul(
                        tri[:, ps * k:ps * k + fi],
                        lhsT=k_h[h][:, s0:s0 + 128],
                        rhs=q_all[h][:, T * g + s0:T * (g + 1)],
                        start=True, stop=True,
                    )
                p_ = pp.tile([128, 1536], F16, tag="p", name="p")
                if ps == fi:
                    nc.scalar.activation(
                        p_[:, 0:np_ * fi], tri[:, 0:np_ * fi], EXP)
                else:
                    nc.scalar.activation(
                        p_[:, 0:np_ * ps].rearrange(
                            "p (k s) -> p k s", k=np_)[:, :, 0:fi],
                        tri[:, 0:np_ * ps].rearrange(
                            "p (k s) -> p k s", k=np_)[:, :, 0:fi],
                        EXP,
                    )

                # e2 = w * p.  Planes with g < NG on gpsimd (per-plane TTs),
                # rest on vector (one wide strided TT).
                e2b = e2p.tile([128, 1536], F16, tag="e2", name="e2")
                nsplit = max(0, min(NG - g0, np_))
                for k in range(nsplit):
                    g = g0 + k
                    nc.gpsimd.tensor_mul(
                        e2b[:, k * fi:(k + 1) * fi],
                        w_s[h][jc][:, g * fi:(g + 1) * fi],
                        p_[:, ps * k:ps * k + fi],
                    )
                if np_ > nsplit:
                    nrun = np_ - nsplit
                    psrc = p_[:, ps * nsplit:ps * np_].rearrange(
                        "p (k s) -> p k s", k=nrun)[:, :, 0:fi]
                    nc.vector.tensor_mul(
                        e2b[:, nsplit * fi:np_ * fi].rearrange(
                            "p (k s) -> p k s", k=nrun),
                        w_s[h][jc][:, (g0 + nsplit) * fi:ghi * fi].rearrange(
                            "p (k s) -> p k s", k=nrun),
                        psrc,
                    )

                pending.append((h, jc, g0, np_, last, e2b))
                if len(pending) > LAG:
                    emit_yt(pending.pop(0))
            for item in pending:
                emit_yt(item)

            # ---------------- output projection ----------------
            for tcc in range(TC):
                o_ps = ps_misc.tile([128, C], F32, tag="misc", name=f"o_ps{tcc}")
                nc.tensor.matmul(
                    o_ps[:],
                    lhsT=yTn[:, 128 * tcc:128 * (tcc + 1)],
                    rhs=wp_s[:],
                    start=True, stop=True,
                )
                o_sb = const.tile([128, C], F32, name=f"o_sb{tcc}")
                nc.scalar.copy(o_sb[:], o_ps[:])
                nc.sync.dma_start(out=d_out[128 * tcc:128 * (tcc + 1), :], in_=o_sb[:])

    nc.compile()
    return nc


def _get_nc():
    if _NC_CACHE[0] is None:
        _NC_CACHE[0] = _build_nc()
    return _NC_CACHE[0]


def _prep_core_inputs(c, x, bm, w_attn, w_proj, w_edge_k, w_edge_v, eet, abt):
    b, hp = divmod(c, 4)
    h0 = 2 * hp
    xT = np.ascontiguousarray(x[b].T).astype(np.float32)            # (C, T)
    triu = np.triu(np.ones((T, T), dtype=bool))                     # j <= i
    bmT = bm[b].T                                                   # bmT[j,i] = bm[b][i,j]
    mm = np.where(triu, bm[b], 255)                                 # (T, T) [j,i]
    ebias = np.exp(abt)                                             # (G, H)
    # masked-bias planes: w[h][j, g, i] = (mm[j,i]==g) * exp(bias[bm[i,j], h])
    wpl = []
    for jc in range(TC):
        s0, fi = 128 * jc, FI[jc]
        wp_ = np.zeros((2, 128, G, fi), np.float16)
        sub = mm[s0:s0 + 128, s0:s0 + fi]
        for h in range(2):
            cbsub = ebias[:, h0 + h][bmT[s0:s0 + 128, s0:s0 + fi]]
            for g in range(G):
                wp_[h, :, g, :] = (sub == g) * cbsub
        wpl.append(np.ascontiguousarray(wp_.reshape(256, G * fi)))
    w_qk = np.concatenate(
        [w_attn[:, hp * 64:(hp + 1) * 64],
         w_attn[:, C + hp * 64:C + (hp + 1) * 64]], axis=1
    ).astype(np.float32)                                            # (C, 128)
    w_v = np.ascontiguousarray(
        w_attn[:, 2 * C + hp * 64:2 * C + (hp + 1) * 64]).astype(np.float32)
    ektab = (eet @ w_edge_k)[:, hp * 64:(hp + 1) * 64] / np.sqrt(E)  # (G, 64)
    ektabT = np.ascontiguousarray(ektab.T).astype(np.float32)        # (64, G)
    evtab = (eet @ w_edge_v)[:, hp * 64:(hp + 1) * 64]               # (G, 64)
    evb = np.zeros((128, G * VW), np.float16)
    for g in range(G):
        evb[:, VW * g:VW * g + 32] = evtab[g, 0:32].astype(np.float16)[None, :]
        evb[:, VW * g + 32] = 1.0
        evb[:, VW * g + 33:VW * g + 65] = evtab[g, 32:64].astype(np.float16)[None, :]
        evb[:, VW * g + 65] = 1.0
    w_proj_sl = np.ascontiguousarray(
        w_proj[hp * 64:(hp + 1) * 64, :]).astype(np.float16)         # (64, C)
    d = {
        "xT": xT, "w_qk": w_qk, "w_v": w_v,
        "ektabT": ektabT, "evb": evb, "w_proj_sl": w_proj_sl,
    }
    for jc in range(TC):
        d[f"wpl{jc}"] = wpl[jc]
    return d


def run(inputs, trace=False):
    x = np.asarray(inputs["x"], np.float32)
    bm = np.asarray(inputs["bias_matrix"]).astype(np.int64)
    w_attn = np.asarray(inputs["w_attn"], np.float32)
    w_proj = np.asarray(inputs["w_proj"], np.float32)
    w_edge_k = np.asarray(inputs["w_edge_k"], np.float32)
    w_edge_v = np.asarray(inputs["w_edge_v"], np.float32)
    eet = np.asarray(inputs["edge_emb_table"], np.float32)
    abt = np.asarray(inputs["attn_bias_table"], np.float32)

    nc = _get_nc()
    in_maps = [
        _prep_core_inputs(c, x, bm, w_attn, w_proj, w_edge_k, w_edge_v, eet, abt)
        for c in range(NCORES)
    ]
    res = run_bass_kernel_spmd(nc, in_maps, core_ids=list(range(NCORES)),
                               trace=trace)
    out = np.zeros((B, T, C), np.float32)
    for c in range(NCORES):
        out[c // 4] += res.results[c]["out"]
    return out, res


def kernel(**inputs) -> np.ndarray:
    out, _ = run(inputs, trace=False)
    return out


# revision 24
# speedup vs baseline: 1.4705x; 1.0064x over previous
"""Edge-conditioned causal self-attention (Graphormer-style) on 8 Trainium2 cores.

Math (per batch b, head h; i = query pos, j = key pos; E=32, G=16 edge types):
  scores[i,j] = sum_e q[i,e] * k[j,e] * ektab[bm[j,i], e] / sqrt(E)
  attn        = softmax_j(scores + biastab[bm[i,j]])  (causal j<=i)
  y[i,e]      = sum_j attn[i,j] * v[j,e] * evtab[bm[j,i], e]
  out         = y @ w_proj
where ektab = edge_emb @ w_edge_k (per-head slice), evtab likewise, and bm is
the integer edge-type matrix.  NOTE: the additive bias indexes bm TRANSPOSED
relative to the k/v modulation (matches the reference einsums).

Since there are only G=16 edge types, scores decompose into 16 per-type Gram
planes S_g = (q .* ektab[g]) @ k^T.  The per-(j,i) selection and the additive
bias are both folded into host-precomputed f16 planes
  w_g[j,i] = (bm[j,i]==g & j<=i) * exp(biastab[bm[i,j]])
so the masked-biased softmax numerators are e2_g = w_g * exp(S_g), and the
output accumulates yT += (v .* evtab[g] | 1) @ e2_g with the ones column
giving the softmax denominator for free.

Schedule (engine-balanced for the TimelineSim cost model):
  - the h0/h1 head streams are interleaved batch-by-batch (shared one-bank
    yT PSUM tile: h0 rows 0-32, h1 rows 64-96) so the PE always has
    independent ready matmuls and stays at full clock;
  - S_g matmuls grouped into 3-bank PSUM tiles, exp'd in one batched strided
    activation (Scalar is the only engine with a cheap PSUM exit);
  - e2 multiplies split: bottom NG g's per batch on GPSIMD (per-plane TTs),
    rest on Vector (one wide strided TT per batch, 2x rate);
  - yT matmuls are emitted LAG batches late so the in-order PE sequencer
    never head-of-line blocks on a dependency-stalled yT matmul.
Sharding: core c -> batch b=c//4, head pair (2*(c%4), 2*(c%4)+1).  Fully
data-parallel, no collectives; host sums the 4 per-core partial projections
per batch (w_proj is row-parallel over head slices).
"""

import numpy as np

import concourse.bass as bass  # noqa: F401
import concourse.mybir as mybir
import concourse.tile as tile
from concourse import bacc
from concourse.bass_utils import run_bass_kernel_spmd

B, T, C, H, E, G = 2, 512, 256, 8, 32, 16
NCORES = 8
EP = E + 1      # 33: v block width per head (32 modulated v cols + ones col)
VW = 2 * EP     # 66: v block for both heads
TC = T // 128   # 4 row chunks

NG = 5          # bottom-NG g's per batch handled on GPSIMD
LAG = 3         # yT matmul emission lag (in batches) on the merged stream
FI = [512, 384, 256, 128]          # i-extent per j-chunk
PSTRIDE = [512, 512, 256, 128]     # psum plane stride per j-chunk
# per-jc PSUM batching: list of (g0, nplanes) with nplanes*PSTRIDE <= 1536
BATCHES = [
    [(0, 3), (3, 3), (6, 3), (9, 3), (12, 3), (15, 1)],
    [(0, 3), (3, 3), (6, 3), (9, 3), (12, 3), (15, 1)],
    [(0, 6), (6, 6), (12, 4)],
    [(0, 12), (12, 4)],
]

F16 = mybir.dt.float16
F32 = mybir.dt.float32
EXP = mybir.ActivationFunctionType.Exp

_NC_CACHE = [None]


def _build_nc():
    nc = bacc.Bacc("TRN2", target_bir_lowering=False)

    d_xT = nc.dram_tensor("xT", (C, T), F16, kind="ExternalInput")
    d_w = [
        nc.dram_tensor(f"wpl{jc}", (256, G * FI[jc]), F16, kind="ExternalInput")
        for jc in range(TC)
    ]
    d_wqk = nc.dram_tensor("w_qk", (C, 128), F16, kind="ExternalInput")
    d_wv = nc.dram_tensor("w_v", (C, 64), F16, kind="ExternalInput")
    d_ekT = nc.dram_tensor("ektabT", (64, G), F32, kind="ExternalInput")
    d_evb = nc.dram_tensor("evb", (128, G * VW), F16, kind="ExternalInput")
    d_wp = nc.dram_tensor("w_proj_sl", (64, C), F16, kind="ExternalInput")
    d_out = nc.dram_tensor("out", (T, C), F32, kind="ExternalOutput")

    with tile.TileContext(nc) as tc:
        with (
            tc.tile_pool(name="const", bufs=1) as const,
            tc.tile_pool(name="ps_misc", bufs=1, space="PSUM") as ps_misc,
            tc.tile_pool(name="ps_tri", bufs=2, space="PSUM") as ps_tri,
            tc.tile_pool(name="ps_y", bufs=1, space="PSUM") as ps_y,
            tc.tile_pool(name="pp", bufs=4) as pp,
            tc.tile_pool(name="e2p", bufs=6) as e2p,
        ):
            # ---------------- input loads ----------------
            xT_s = []
            for i in range(2):
                t_ = const.tile([128, T], F16, name=f"xT{i}")
                nc.sync.dma_start(out=t_[:], in_=d_xT[128 * i:128 * (i + 1), :])
                xT_s.append(t_)
            wqk_s = []
            for i in range(2):
                t_ = const.tile([128, 128], F16, name=f"wqk{i}")
                nc.sync.dma_start(out=t_[:], in_=d_wqk[128 * i:128 * (i + 1), :])
                wqk_s.append(t_)
            wv_s = []
            for i in range(2):
                t_ = const.tile([128, 64], F16, name=f"wv{i}")
                nc.sync.dma_start(out=t_[:], in_=d_wv[128 * i:128 * (i + 1), :])
                wv_s.append(t_)
            ekT_s = const.tile([64, G], F32, name="ekT")
            nc.sync.dma_start(out=ekT_s[:], in_=d_ekT[:, :])
            evb_s = const.tile([128, G * VW], F16, name="evb")
            nc.sync.dma_start(out=evb_s[:], in_=d_evb[:, :])
            wp_s = const.tile([64, C], F16, name="wp")
            nc.sync.dma_start(out=wp_s[:], in_=d_wp[:, :])
            # masked-bias planes, in processing order (jc-major, h inner),
            # two half-tile DMAs each so early batches unblock sooner
            w_s = [[None] * TC, [None] * TC]
            for jc in range(TC):
                half = G * FI[jc] // 2
                for h in range(2):
                    t_ = const.tile([128, G * FI[jc]], F16, name=f"w{h}_{jc}")
                    nc.sync.dma_start(
                        out=t_[:, 0:half],
                        in_=d_w[jc][128 * h:128 * (h + 1), 0:half])
                    nc.sync.dma_start(
                        out=t_[:, half:],
                        in_=d_w[jc][128 * h:128 * (h + 1), half:])
                    w_s[h][jc] = t_

            # ---------------- q/k/v projections ----------------
            # qk_ps rows: 0-31 qT(h0), 32-63 qT(h1), 64-95 kT(h0), 96-127 kT(h1)
            qk_ps = ps_misc.tile([128, T], F32, tag="misc", name="qk_ps")
            for ck in range(2):
                nc.tensor.matmul(
                    qk_ps[:],
                    lhsT=wqk_s[ck][:],
                    rhs=xT_s[ck][:],
                    start=(ck == 0), stop=(ck == 1),
                )
            qk_sb = const.tile([128, T], F16, name="qk_sb")
            nc.vector.tensor_copy(out=qk_sb[:], in_=qk_ps[:])

            # v_ps cols per t-chunk: 32 v(h0) | 32 v(h1)
            v_ps = ps_misc.tile([128, TC * 64], F32, tag="misc", name="v_ps")
            for tcc in range(TC):
                for ck in range(2):
                    nc.tensor.matmul(
                        v_ps[:, 64 * tcc:64 * (tcc + 1)],
                        lhsT=xT_s[ck][:, 128 * tcc:128 * (tcc + 1)],
                        rhs=wv_s[ck][:],
                        start=(ck == 0), stop=(ck == 1),
                    )

            # k tiles (contraction layout: e on partitions, j on free)
            k_h = []
            for h in range(2):
                t_ = const.tile([E, T], F16, name=f"k{h}")
                nc.vector.tensor_copy(out=t_[:], in_=qk_sb[64 + 32 * h:96 + 32 * h, :])
                k_h.append(t_)

            # q' tiles: per g block of T cols, rows = e, modulated by
            # ektab/sqrt(E).  Built lazily inside the main loop so the
            # in-order vector queue doesn't block early e2 work.
            q_all = []
            for h in range(2):
                t_ = const.tile([E, G * T], F16, name=f"qall{h}")
                q_all.append(t_)
            q_built = set()

            def build_q(h, g):
                if (h, g) in q_built:
                    return
                q_built.add((h, g))
                nc.vector.tensor_scalar_mul(
                    q_all[h][:, T * g:T * (g + 1)],
                    qk_sb[32 * h:32 * h + 32, :],
                    ekT_s[32 * h:32 * h + 32, g:g + 1],
                )

            # v66 per t-chunk: [v_h0(32) | 1 | v_h1(32) | 1]
            v66 = []
            for tcc in range(TC):
                t_ = const.tile([128, VW], F16, name=f"v66_{tcc}")
                nc.vector.tensor_copy(out=t_[:, 0:32], in_=v_ps[:, 64 * tcc:64 * tcc + 32])
                nc.vector.tensor_copy(out=t_[:, 33:65], in_=v_ps[:, 64 * tcc + 32:64 * tcc + 64])
                nc.vector.memset(t_[:, 32:33], 1.0)
                nc.vector.memset(t_[:, 65:66], 1.0)
                v66.append(t_)

            # v'_g = v66 * evtab broadcast: one wide TT per t-chunk (lazy)
            vall = []
            for tcc in range(TC):
                t_ = const.tile([128, G * VW], F16, name=f"vall{tcc}")
                vall.append(t_)
            vall_built = set()

            def build_vall(tcc):
                if tcc in vall_built:
                    return
                vall_built.add(tcc)
                nc.vector.tensor_mul(
                    vall[tcc][:].rearrange("p (g w) -> p g w", g=G),
                    v66[tcc][:].unsqueeze(1).broadcast_to([128, G, VW]),
                    evb_s[:].rearrange("p (g w) -> p g w", g=G),
                )

            ones32 = const.tile([1, 32], F32, name="ones32")
            nc.vector.memset(ones32[:], 1.0)
            yTn = const.tile([64, T], F16, name="yTn")

            # ---------------- main attention loops ----------------
            # One shared PSUM bank holds both heads' accumulators:
            # h0 rows 0-32, h1 rows 64-96.
            yTT = ps_y.tile([64 + EP, T], F32, tag="y", name="yTT")
            first_h = [True, True]

            def emit_norm(h):
                recip = const.tile([1, T], F32, name=f"recip{h}")
                nc.vector.reciprocal(recip[:], yTT[64 * h + 32:64 * h + 33, :])
                bc_ps = ps_misc.tile([32, T], F32, tag="misc", name=f"bc{h}")
                nc.tensor.matmul(bc_ps[:], lhsT=ones32[:], rhs=recip[:],
                                 start=True, stop=True)
                rb_sb = const.tile([32, T], F32, name=f"rb{h}")
                nc.vector.tensor_copy(out=rb_sb[:], in_=bc_ps[:])
                nc.vector.tensor_mul(yTn[32 * h:32 * (h + 1), :],
                                     yTT[64 * h:64 * h + 32, :], rb_sb[:])

            def emit_yt(item):
                h, jc, g0, np_, last, e2b = item
                fi = FI[jc]
                s0 = 128 * jc
                for k in range(np_):
                    g = g0 + k
                    nc.tensor.matmul(
                        yTT[64 * h:64 * h + EP, s0:s0 + fi],
                        lhsT=vall[jc][:, VW * g + EP * h:VW * g + EP * (h + 1)],
                        rhs=e2b[:, k * fi:(k + 1) * fi],
                        start=first_h[h],
                        stop=(last and k == np_ - 1),
                    )
                    first_h[h] = False
                if last:
                    emit_norm(h)

            flat = []
            for jc in range(TC):
                for bi, (g0, np_) in enumerate(BATCHES[jc]):
                    for h in range(2):
                        flat.append((h, jc, g0, np_,
                                     bi == len(BATCHES[jc]) - 1 and jc == TC - 1))

            pending = []
            for (h, jc, g0, np_, last) in flat:
                s0 = 128 * jc
                fi = FI[jc]
                ps = PSTRIDE[jc]
                ghi = g0 + np_

                build_vall(jc)
                for k in range(np_):
                    build_q(h, g0 + k)
                tri = ps_tri.tile([128, 1536], F32, tag="tri", name="tri")
                for k in range(np_):
                    g = g0 + k
                    nc.tensor.matmul(
                        tri[:, ps * k:ps * k + fi],
                        lhsT=k_h[h][:, s0:s0 + 128],
                        rhs=q_all[h][:, T * g + s0:T * (g + 1)],
                        start=True, stop=True,
                    )
                p_ = pp.tile([128, 1536], F16, tag="p", name="p")
                if ps == fi:
                    nc.scalar.activation(
                        p_[:, 0:np_ * fi], tri[:, 0:np_ * fi], EXP)
                else:
                    nc.scalar.activation(
                        p_[:, 0:np_ * ps].rearrange(
                            "p (k s) -> p k s", k=np_)[:, :, 0:fi],
                        tri[:, 0:np_ * ps].rearrange(
                            "p (k s) -> p k s", k=np_)[:, :, 0:fi],
                        EXP,
                    )

                # e2 = w * p.  Planes with g < NG on gpsimd (per-plane TTs),
                # rest on vector (one wide strided TT).
                e2b = e2p.tile([128, 1536], F16, tag="e2", name="e2")
                nsplit = max(0, min(NG - g0, np_))
                for k in range(nsplit):
                    g = g0 + k
                    nc.gpsimd.tensor_mul(
                        e2b[:, k * fi:(k + 1) * fi],
                        w_s[h][jc][:, g * fi:(g + 1) * fi],
                        p_[:, ps * k:ps * k + fi],
                    )
                if np_ > nsplit:
                    nrun = np_ - nsplit
                    psrc = p_[:, ps * nsplit:ps * np_].rearrange(
                        "p (k s) -> p k s", k=nrun)[:, :, 0:fi]
                    nc.vector.tensor_mul(
                        e2b[:, nsplit * fi:np_ * fi].rearrange(
                            "p (k s) -> p k s", k=nrun),
                        w_s[h][jc][:, (g0 + nsplit) * fi:ghi * fi].rearrange(
                            "p (k s) -> p k s", k=nrun),
                        psrc,
                    )

                pending.append((h, jc, g0, np_, last, e2b))
                if len(pending) > LAG:
                    emit_yt(pending.pop(0))
            for item in pending:
                emit_yt(item)

            # ---------------- output projection ----------------
            for tcc in range(TC):
                o_ps = ps_misc.tile([128, C], F32, tag="misc", name=f"o_ps{tcc}")
                nc.tensor.matmul(
                    o_ps[:],
                    lhsT=yTn[:, 128 * tcc:128 * (tcc + 1)],
                    rhs=wp_s[:],
                    start=True, stop=True,
                )
                o_sb = const.tile([128, C], F32, name=f"o_sb{tcc}")
                nc.vector.tensor_copy(out=o_sb[:], in_=o_ps[:])
                nc.sync.dma_start(out=d_out[128 * tcc:128 * (tcc + 1), :], in_=o_sb[:])

    nc.compile()
    return nc


def _get_nc():
    if _NC_CACHE[0] is None:
        _NC_CACHE[0] = _build_nc()
    return _NC_CACHE[0]


def _prep_core_inputs(c, x, bm, w_attn, w_proj, w_edge_k, w_edge_v, eet, abt):
    b, hp = divmod(c, 4)
    h0 = 2 * hp
    xT = np.ascontiguousarray(x[b].T).astype(np.float16)            # (C, T)
    triu = np.triu(np.ones((T, T), dtype=bool))                     # j <= i
    bmT = bm[b].T                                                   # bmT[j,i] = bm[b][i,j]
    mm = np.where(triu, bm[b], 255)                                 # (T, T) [j,i]
    ebias = np.exp(abt)                                             # (G, H)
    # masked-bias planes: w[h][j, g, i] = (mm[j,i]==g) * exp(bias[bm[i,j], h])
    wpl = []
    for jc in range(TC):
        s0, fi = 128 * jc, FI[jc]
        wp_ = np.zeros((2, 128, G, fi), np.float16)
        sub = mm[s0:s0 + 128, s0:s0 + fi]
        for h in range(2):
            cbsub = ebias[:, h0 + h][bmT[s0:s0 + 128, s0:s0 + fi]]
            for g in range(G):
                wp_[h, :, g, :] = (sub == g) * cbsub
        wpl.append(np.ascontiguousarray(wp_.reshape(256, G * fi)))
    w_qk = np.concatenate(
        [w_attn[:, hp * 64:(hp + 1) * 64],
         w_attn[:, C + hp * 64:C + (hp + 1) * 64]], axis=1
    ).astype(np.float16)                                            # (C, 128)
    w_v = np.ascontiguousarray(
        w_attn[:, 2 * C + hp * 64:2 * C + (hp + 1) * 64]).astype(np.float16)
    ektab = (eet @ w_edge_k)[:, hp * 64:(hp + 1) * 64] / np.sqrt(E)  # (G, 64)
    ektabT = np.ascontiguousarray(ektab.T).astype(np.float32)        # (64, G)
    evtab = (eet @ w_edge_v)[:, hp * 64:(hp + 1) * 64]               # (G, 64)
    evb = np.zeros((128, G * VW), np.float16)
    for g in range(G):
        evb[:, VW * g:VW * g + 32] = evtab[g, 0:32].astype(np.float16)[None, :]
        evb[:, VW * g + 32] = 1.0
        evb[:, VW * g + 33:VW * g + 65] = evtab[g, 32:64].astype(np.float16)[None, :]
        evb[:, VW * g + 65] = 1.0
    w_proj_sl = np.ascontiguousarray(
        w_proj[hp * 64:(hp + 1) * 64, :]).astype(np.float16)         # (64, C)
    d = {
        "xT": xT, "w_qk": w_qk, "w_v": w_v,
        "ektabT": ektabT, "evb": evb, "w_proj_sl": w_proj_sl,
    }
    for jc in range(TC):
        d[f"wpl{jc}"] = wpl[jc]
    return d


def run(inputs, trace=False):
    x = np.asarray(inputs["x"], np.float32)
    bm = np.asarray(inputs["bias_matrix"]).astype(np.int64)
    w_attn = np.asarray(inputs["w_attn"], np.float32)
    w_proj = np.asarray(inputs["w_proj"], np.float32)
    w_edge_k = np.asarray(inputs["w_edge_k"], np.float32)
    w_edge_v = np.asarray(inputs["w_edge_v"], np.float32)
    eet = np.asarray(inputs["edge_emb_table"], np.float32)
    abt = np.asarray(inputs["attn_bias_table"], np.float32)

    nc = _get_nc()
    in_maps = [
        _prep_core_inputs(c, x, bm, w_attn, w_proj, w_edge_k, w_edge_v, eet, abt)
        for c in range(NCORES)
    ]
    res = run_bass_kernel_spmd(nc, in_maps, core_ids=list(range(NCORES)),
                               trace=trace)
    out = np.zeros((B, T, C), np.float32)
    for c in range(NCORES):
        out[c // 4] += res.results[c]["out"]
    return out, res


def kernel(**inputs) -> np.ndarray:
    out, _ = run(inputs, trace=False)
    return out
